# revision 60
# baseline (speedup 1.0000x reference)
"""Causal self-attention (dense transformer block) for 8 Trainium2 NeuronCores.

Sharding: DP over batch (2) x TP over heads (4 groups of 4 heads) = 8 cores.
Per core: column-parallel QKV projection (4 heads), RoPE, causal
flash-attention (no-max-subtraction softmax with constant bias), row-parallel
output projection producing a partial [oc, t] result in fp16; host sums the
4 TP partials per batch and transposes back.

v2 design vs baseline:
- bf16 storage for x/w/q/k/v/yt/p (PSUM accumulation stays f32): halves DMA
  and SBUF, so x stays resident across both head-pairs (no re-read, no yt
  DRAM spill) and the output DMA is fp16.
- Softmax denominators: p tiles reduce via pair/quad DVE adds (independent
  bf16 2x ops) + one short ones-matmul per pair/quad, instead of a full
  512-cycle PE pass per kt tile.
- RoPE's rotate-by-64-partitions runs as two SBUF->SBUF DMAs on idle DMA
  engines instead of a PE permutation matmul.
- Fine-grained causal diagonal: the last two kt tiles of each diagonal block
  compute only q columns 256:512 (the rest is fully masked).
- Phase interleaving: pair-1 QKV matmuls are emitted between pair-0 attention
  blocks, and the output projection trails pair-1 attention by one q-group,
  so scalar-engine exps always hide under dense PE matmul streams.
- Startup: warmup matmuls run on memset tiles (no DMA dependency), input DMAs
  are issued in fine (w,x) chunk pairs consumed by a cc-major first t-group,
  with keepalive matmuls keeping the PE's HAM clock-gate warm through the
  DMA ramp.

Self-contained: hardcodes shapes, builds/compiles/runs the Bass kernel via
run_bass_kernel_spmd on cores 0-7.
"""

import os
import sys
import types

sys.path.insert(0, "/opt/trn_rl_repo")

import numpy as np
import ml_dtypes

import concourse.bass as bass
import concourse.mybir as mybir
import concourse.tile as tile
from concourse import bacc
from concourse.bass_utils import run_bass_kernel_spmd
from concourse.vector_clock import ScopedClock, VectorClock

F32 = mybir.dt.float32
F32R = mybir.dt.float32r
BF16 = mybir.dt.bfloat16
F16 = mybir.dt.float16
AF = mybir.ActivationFunctionType
ALU = mybir.AluOpType

P = 128
T = 2048
C = 2048
NH = 16          # total heads
HPC = 4          # heads per core
HSIZE = 128
N_CORES = 8
TG = 4           # t-groups of 512
QG = 512
EXP_BIAS = -10.0
SCALE = 1.0 / float(np.sqrt(HSIZE))

_TRACE = os.environ.get("BASS_KERNEL_TRACE", "0") == "1"


def _patch_tile_drain():
    """walrus in this toolchain allows at most one sync-wait per instruction;
    TileContext's tail drain aggregates the whole global clock onto one Drain.
    Split it: one Drain per pending proc, each with a single wait."""
    if getattr(tile.TileContext, "_drain_patched", False):
        return

    def _drain_and_barrier(self, tick_clock, wait_clock):
        nc = self.nc
        gc = tick_clock.global_clock
        n = len(gc)
        for p in range(n):
            if gc[p] > 0:
                vc = VectorClock([gc[p] if i == p else 0 for i in range(n)])
                di = nc.sync.drain()
                wait_clock.add_sem_waits(di.ins, ScopedClock({None: vc}))
        nc.all_engine_barrier()
        popped = nc._tile_sem_poison_stack.pop()
        assert popped is self._sem_poison
        nc.clear_and_free_semaphores(list(self.sems.allocated().values()))
        nc.all_engine_barrier()

    tile.TileContext._drain_and_barrier = _drain_and_barrier
    tile.TileContext._drain_patched = True


def _install_ntff_hook():
    """Wire the axon NTFF profiling hook this image leaves unwired (the agent
    image's antenv lacks axon_hooks). Only needed when tracing."""
    import antenv

    if getattr(antenv, "axon_hooks", None) is not None:
        return
    mod = types.ModuleType("antenv.axon_hooks")
    mod._hook = None
    mod.set_axon_ntff_profile_hook = lambda h: setattr(mod, "_hook", h)
    mod.get_axon_ntff_profile_hook = lambda: mod._hook
    sys.modules["antenv.axon_hooks"] = mod
    antenv.axon_hooks = mod
    if "/root/.axon_site" not in sys.path:
        sys.path.insert(0, "/root/.axon_site")
    try:
        from trn_agent_boot.trn_boot import _ntff_profile_via_ctypes

        hook = _ntff_profile_via_ctypes("/opt/axon/libaxon_pjrt.so")
        if hook is not None:
            mod.set_axon_ntff_profile_hook(hook)
        import concourse.bass_utils as bu

        bu.upload_artifacts = lambda d: d
    except Exception:
        pass


def build_nc():
    _patch_tile_drain()
    nc = bacc.Bacc(None, target_bir_lowering=False)

    xT = nc.dram_tensor("xT", [C, T], BF16, kind="ExternalInput")
    w = nc.dram_tensor("w", [C, 6 * HSIZE * 2], BF16, kind="ExternalInput")
    wp = nc.dram_tensor("wp", [HPC * HSIZE, T], BF16, kind="ExternalInput")
    c1d = nc.dram_tensor("c1", [P, T], BF16, kind="ExternalInput")
    c2d = nc.dram_tensor("c2", [P, T], BF16, kind="ExternalInput")
    mkd = nc.dram_tensor("mk", [2, P, QG], BF16, kind="ExternalInput")
    onesd = nc.dram_tensor("ones_col", [P, 1], BF16, kind="ExternalInput")
    outT = nc.dram_tensor("outT", [T, T], F16, kind="ExternalOutput")  # [oc, t]

    xTr = xT.rearrange("(cc p) t -> p cc t", p=P)      # [128,16,2048]
    wr = w.rearrange("(cc p) j -> p cc j", p=P)        # [128,16,1536]
    wpr = wp.rearrange("(hc p) t -> p hc t", p=P)      # [128,4,2048]
    mkr = mkd.rearrange("s p q -> p s q")              # [128,2,512]

    with tile.TileContext(nc) as tc, nc.allow_low_precision(
        reason="bf16 storage / f32 accumulation is the intended format"
    ):
        with (
            tc.tile_pool(name="const", bufs=1) as constp,
            tc.tile_pool(name="xpool", bufs=1) as xpool,
            tc.tile_pool(name="wpool", bufs=1) as wpool,
            tc.tile_pool(name="qk", bufs=1) as qkres,
            tc.tile_pool(name="vres", bufs=1) as vresp,
            tc.tile_pool(name="yres", bufs=1) as yresp,
            tc.tile_pool(name="work", bufs=5) as work,
            tc.tile_pool(name="pwork", bufs=6) as pwork,
            tc.tile_pool(name="ppp", bufs=2) as pppool,
            tc.tile_pool(name="yp", bufs=2) as yprep,
            tc.tile_pool(name="rp", bufs=2) as rpool,
            tc.tile_pool(name="rbig", bufs=1) as rbigp,
            tc.tile_pool(name="stg", bufs=6) as stgp,
            tc.tile_pool(name="mm", bufs=3, space="PSUM") as mmp,
            tc.tile_pool(name="yt", bufs=2, space="PSUM") as ytp,
            tc.tile_pool(name="lp", bufs=1, space="PSUM") as lpp,
        ):
            # ---- resident tiles ----
            x_sb = xpool.tile([P, 16, T], BF16, tag="x")          # 8.39MB
            c1 = constp.tile([P, T], BF16, tag="c1")
            c2 = constp.tile([P, T], BF16, tag="c2")
            mk = constp.tile([P, 2, QG], BF16, tag="mk")
            ones_c = constp.tile([P, 1], BF16, tag="onc")
            ebias = constp.tile([P, 1], F32, tag="ebias")
            wu_st = constp.tile([P, P], BF16, tag="wu_st")
            wu_mv = constp.tile([P, 128], BF16, tag="wu_mv")
            # q/k per pair per head: separate tags so pair1 QKV never waits
            # on pair0 attention reads
            q_sb = [[qkres.tile([P, T], BF16, tag=f"q{pr}{h}", name=f"q{pr}{h}")
                     for h in range(2)] for pr in range(2)]
            k_sb = [[qkres.tile([P, T], BF16, tag=f"k{pr}{h}", name=f"k{pr}{h}")
                     for h in range(2)] for pr in range(2)]
            # v per pair: pair-1 v writes must not alias pair-0's values
            # while pair-0 attention is still reading them
            v_sb = [vresp.tile([P, 16, 256], BF16, tag=f"v{pr}", name=f"v{pr}")
                    for pr in range(2)]
            yt_sb = [yresp.tile([P, T], BF16, tag=f"yt{i}", name=f"yt{i}")
                     for i in range(4)]

            nc.vector.memset(ebias[:], EXP_BIAS)
            nc.vector.memset(wu_st[:], 0.125)
            nc.vector.memset(wu_mv[:], 0.125)

            # ---- PE warmup on memset tiles: no DMA dependency ----
            for i in range(42):
                ps_wu = mmp.tile([P, 128], F32, tag="mm", name="ps_wu")
                nc.tensor.matmul(ps_wu[:], wu_st[:], wu_mv[:],
                                 start=True, stop=True)

            def keepalive(ap128):
                """One tiny matmul reading a landed DMA chunk: keeps the HAM
                activity window busy during the startup DMA ramp."""
                ps_ka = mmp.tile([P, P], F32, tag="mm", name="ps_ka")
                nc.tensor.matmul(ps_ka[:], wu_st[:], ap128, start=True,
                                 stop=True)

            # ---- startup DMAs: per-cc-chunk (w, x) interleaved so the
            # cc-major tg0 QKV unlocks 8 matmuls per landed chunk ----
            w_sb = wpool.tile([P, 16, 768], BF16, tag="w", name="w_sb0")
            for ch in range(8):
                nc.sync.dma_start(
                    w_sb[:, ch * 2:(ch + 1) * 2, :],
                    wr[:, ch * 2:(ch + 1) * 2, 0:768],
                )
                keepalive(w_sb[:, ch * 2, 0:128])
                nc.sync.dma_start(
                    x_sb[:, ch * 2:(ch + 1) * 2, 0:QG],
                    xTr[:, ch * 2:(ch + 1) * 2, 0:QG],
                )
                keepalive(x_sb[:, ch * 2, 0:128])
            nc.sync.dma_start(c1[:], c1d[:])
            nc.sync.dma_start(c2[:], c2d[:])
            for tg in range(1, TG):
                for ch in range(4):
                    nc.sync.dma_start(
                        x_sb[:, ch * 4:(ch + 1) * 4, tg * QG:(tg + 1) * QG],
                        xTr[:, ch * 4:(ch + 1) * 4, tg * QG:(tg + 1) * QG],
                    )
                if tg == 1:
                    keepalive(x_sb[:, 0, QG:QG + 128])
            nc.sync.dma_start(ones_c[:], onesd[:])
            nc.sync.dma_start(mk[:], mkr)
            # proj weights: own tag, loaded up front so phase C never waits
            wp_sb = wpool.tile([P, 4, T], BF16, tag="wpj", name="wp_sb")
            for ocq in range(4):
                nc.sync.dma_start(
                    wp_sb[:, :, ocq * 512:(ocq + 1) * 512],
                    wpr[:, :, ocq * 512:(ocq + 1) * 512],
                )

            # ---------- building blocks ----------

            def rope(pair, j, psum, tg):
                """psum [128,512] raw q/k j-tile -> roped bf16 into q/k_sb.
                j: 0,1 = q heads 0,1; 2,3 = k heads 0,1. The rotate-by-64
                partition swap runs as two SBUF->SBUF DMAs (idle DMA engines)
                instead of a PE permutation matmul; DVE ops stay pure-bf16."""
                dst = (q_sb[pair] if j < 2 else k_sb[pair])[j % 2]
                dsl = dst[:, tg * QG:(tg + 1) * QG]
                qraw = work.tile([P, QG], BF16, tag="tmp", name="qraw")
                nc.scalar.activation(qraw[:], psum[:], AF.Copy)
                qs = work.tile([P, QG], BF16, tag="tmp", name="qs")
                nc.scalar.dma_start(qs[0:64, :], qraw[64:128, :])
                nc.scalar.dma_start(qs[64:128, :], qraw[0:64, :])
                t1 = work.tile([P, QG], BF16, tag="tmp", name="t1")
                t2 = work.tile([P, QG], BF16, tag="tmp", name="t2")
                c1s = c1[:, tg * QG:(tg + 1) * QG]
                c2s = c2[:, tg * QG:(tg + 1) * QG]
                nc.vector.tensor_mul(t1[:], qraw[:], c1s)
                nc.vector.tensor_mul(t2[:], qs[:], c2s)
                nc.vector.tensor_add(dsl, t1[:], t2[:])

            def qkv_jp_unit(pair, tg, jp):
                """One (tg, jp) QKV unit: 2 psum accumulations over 16 cc,
                then rope both j-tiles."""
                psq = [mmp.tile([P, QG], F32, tag="psq", bufs=2,
                                name=f"psq{j}")
                       for j in range(2)]
                for cc in range(16):
                    xs = x_sb[:, cc, tg * QG:(tg + 1) * QG]
                    for j in range(2):
                        nc.tensor.matmul(
                            psq[j][:],
                            w_sb[:, cc, (jp * 2 + j) * 128:(jp * 2 + j + 1) * 128],
                            xs,
                            start=(cc == 0),
                            stop=(cc == 15),
                        )
                rope(pair, jp * 2 + 0, psq[0], tg)
                rope(pair, jp * 2 + 1, psq[1], tg)

            def qkv_v_unit(pair, tg):
                """v for one tg: 4 t-tiles, N=256 (both heads)."""
                for tt in range(4):
                    psv = ytp.tile([P, 256], F32, tag="yt", name="psv")
                    for cc in range(16):
                        nc.tensor.matmul(
                            psv[:],
                            x_sb[:, cc, (tg * 4 + tt) * 128:(tg * 4 + tt + 1) * 128],
                            w_sb[:, cc, 512:768],
                            start=(cc == 0),
                            stop=(cc == 15),
                        )
                    nc.vector.tensor_copy(v_sb[pair][:, tg * 4 + tt, :],
                                          psv[:])

            def attn_block(pair, h, qg):
                """Flash-attention block for one (head, q-group). Softmax
                denominator: pairwise p-tile adds on DVE (independent bf16
                2x ops) + one small ones-matmul per pair of kt tiles."""
                hg = pair * 2 + h
                n_kt = 4 * qg + 4
                LA = 3
                ps_y = ytp.tile([P, QG], F32, tag="yt", name="ps_y")
                ps_l = lpp.tile([1, QG], F32, tag="l", name="ps_l")
                p_tiles = {}
                pp_pend = [None]
                lfirst = [True]

                def emit_s(kt):
                    s = kt - 4 * qg
                    # last two diagonal tiles: q cols [0,256) are entirely
                    # masked, so compute only the upper half width
                    half = s >= 2
                    qo = 256 if half else 0
                    w = 256 if half else QG
                    ps_s = mmp.tile([P, w], F32, tag="mm", name="ps_s")
                    nc.tensor.matmul(
                        ps_s[:],
                        k_sb[pair][h][:, kt * 128:(kt + 1) * 128],
                        q_sb[pair][h][:, qg * QG + qo:qg * QG + qo + w],
                        start=True,
                        stop=True,
                    )
                    p_sb = pwork.tile([P, w], BF16, tag="p", name="p_sb")
                    nc.scalar.activation(
                        p_sb[:], ps_s[:], AF.Exp, bias=ebias[:], scale=SCALE
                    )
                    if s >= 0:
                        # half tiles (s=2,3) at q cols 256:512 follow the
                        # same triangular pattern as s-2 at cols 0:256
                        ms, mo = (s - 2, 0) if half else (s, qo)
                        nc.vector.tensor_mul(p_sb[:], p_sb[:],
                                             mk[:, ms, mo:mo + w])
                    p_tiles[kt] = p_sb
                    if kt % 2 == 1:
                        # pair partner always has the same width: the two
                        # half tiles are the final (even, odd) kt pair
                        pp = pppool.tile([P, w], BF16, tag="pp", name="pp")
                        nc.vector.tensor_add(
                            pp[:], p_tiles[kt - 1][:], p_sb[:]
                        )
                        # full-width pairs combine into quads (one DVE add
                        # replaces a PE ones-matmul); the final mixed-width
                        # quad keeps two matmuls
                        if w == QG and kt % 4 == 1 and kt + 2 < n_kt - 1:
                            pp_pend[0] = pp
                            return
                        if w == QG and kt % 4 == 3 and pp_pend[0] is not None:
                            nc.vector.tensor_add(pp[:], pp_pend[0][:], pp[:])
                            pp_pend[0] = None
                        nc.tensor.matmul(
                            ps_l[0:1, qo:qo + w], ones_c[:], pp[:],
                            start=lfirst[0], stop=(kt == n_kt - 1),
                            skip_group_check=True,
                        )
                        lfirst[0] = False

                def emit_av(kt):
                    half = (kt - 4 * qg) >= 2
                    qo = 256 if half else 0
                    w = 256 if half else QG
                    p_sb = p_tiles.pop(kt)
                    nc.tensor.matmul(
                        ps_y[:, qo:qo + w],
                        v_sb[pair][:, kt, h * 128:(h + 1) * 128],
                        p_sb[:],
                        start=(kt == 0),
                        stop=(kt == n_kt - 1),
                        skip_group_check=True,
                    )

                for kt in range(n_kt + LA):
                    if kt < n_kt:
                        emit_s(kt)
                    if kt >= LA:
                        emit_av(kt - LA)

                # free ps_y immediately; normalize later in pure bf16
                ypre = yprep.tile([P, QG], BF16, tag="ypre", name="ypre")
                nc.vector.tensor_copy(ypre[:], ps_y[:])
                with tc.high_priority(offset=40):
                    r_f32 = rpool.tile([1, QG], F32, tag="rf", name="r_f32")
                    nc.vector.reciprocal_approx_fast(r_f32[:], ps_l[:])
                    r_bf = rpool.tile([1, QG], BF16, tag="rb", name="r_bf")
                    nc.vector.tensor_copy(r_bf[:], r_f32[:])
                r128 = rbigp.tile([P, QG], BF16, tag="r128", name="r128")
                nc.gpsimd.partition_broadcast(r128[:], r_bf[0:1, :])
                nc.vector.tensor_mul(
                    yt_sb[hg][:, qg * QG:(qg + 1) * QG], ypre[:], r128[:]
                )

            def proj_chunk(tg, oc_lo, oc_hi, alt=False):
                """Output projection tiles [oc_lo, oc_hi) for one t-group.
                oc tiles run in pairs (two concurrent psum groups) so each
                psum slot cycle carries ~1.7us of PE work, hiding the
                stage-copy latency; stage copies alternate scalar/DVE.
                alt=True (proj-only tail): alternate pair-groups between the
                psq and mm tags for 5 effective slots."""
                for gi, ocp in enumerate(range(oc_lo // 2, oc_hi // 2)):
                    if alt and gi % 2 == 1:
                        ps_o = [mmp.tile([P, QG], F32, tag="mm",
                                         name=f"ps_o{j}") for j in range(2)]
                    else:
                        ps_o = [mmp.tile([P, QG], F32, tag="psq", bufs=2,
                                         name=f"ps_o{j}") for j in range(2)]
                    for hc in range(4):
                        for j in range(2):
                            oc = ocp * 2 + j
                            nc.tensor.matmul(
                                ps_o[j][:],
                                wp_sb[:, hc, oc * 128:(oc + 1) * 128],
                                yt_sb[hc][:, tg * QG:(tg + 1) * QG],
                                start=(hc == 0),
                                stop=(hc == 3),
                            )
                    for j in range(2):
                        oc = ocp * 2 + j
                        stage = stgp.tile([P, QG], F16, tag="stg",
                                          name="stage")
                        if j == 0:
                            nc.scalar.activation(stage[:], ps_o[j][:],
                                                 AF.Copy)
                        else:
                            nc.vector.tensor_copy(stage[:], ps_o[j][:])
                        # spread out-DMAs over both HWDGE queues
                        dma_eng = nc.sync if j == 0 else nc.scalar
                        dma_eng.dma_start(
                            outT[oc * 128:(oc + 1) * 128,
                                 tg * QG:(tg + 1) * QG],
                            stage[:],
                        )

            # ---------- phase A: QKV pair 0 ----------
            # tg0 runs cc-major (all 6 psum groups at once) so PE work
            # unlocks progressively as each startup (w,x) chunk lands
            psq4 = [mmp.tile([P, QG], F32, tag="psq", bufs=2,
                             name=f"psq4_{j}") for j in range(2)]
            psq4 += [mmp.tile([P, QG], F32, tag="mm", name=f"psq4_{j}")
                     for j in range(2, 4)]
            for cc in range(16):
                xs = x_sb[:, cc, 0:QG]
                for j in range(4):
                    nc.tensor.matmul(
                        psq4[j][:], w_sb[:, cc, j * 128:(j + 1) * 128], xs,
                        start=(cc == 0), stop=(cc == 15),
                    )
            for j in range(4):
                rope(0, j, psq4[j], 0)
            qkv_v_unit(0, 0)
            for tg in range(1, TG):
                qkv_jp_unit(0, tg, 0)
                qkv_jp_unit(0, tg, 1)
                qkv_v_unit(0, tg)

            # pair-1 weights land during phase B (fine chunks so the first
            # QKV units' cc-consumption tracks DMA arrival)
            w_sb = wpool.tile([P, 16, 768], BF16, tag="w", name="w_sb1")
            for wcc in range(8):
                nc.sync.dma_start(
                    w_sb[:, wcc * 2:(wcc + 1) * 2, :],
                    wr[:, wcc * 2:(wcc + 1) * 2, 768:1536],
                )

            # ---------- phase B: attention pair 0 || QKV pair 1 ----------
            # attention blocks first while pair-1 weights land; QKV units
            # interleaved to keep PE fed while scalar runs exps
            b_sched = [
                ("a", 0, 0), ("a", 1, 0), ("a", 0, 1), ("a", 1, 1),
                ("q", 0, 0), ("v", 0, None), ("q", 1, 0), ("a", 0, 2),
                ("q", 2, 0), ("a", 1, 2), ("q", 3, 0), ("a", 0, 3),
                ("q", 0, 1), ("A", 0, 0), ("q", 1, 1), ("v", 1, None),
                ("A", 1, 0), ("q", 2, 1), ("a", 1, 3), ("q", 3, 1),
                ("v", 2, None), ("v", 3, None),
            ]
            for kind, i1, i2 in b_sched:
                if kind == "a":
                    attn_block(0, i1, i2)
                elif kind == "A":
                    attn_block(1, i1, i2)
                elif kind == "q":
                    qkv_jp_unit(1, i1, i2)
                else:
                    qkv_v_unit(1, i1)

            # ---------- phase C: attention pair 1 || projection ----------
            # proj chunks trail attention by one q-group so the final attn
            # blocks always have dense proj matmuls co-scheduled behind
            # their scalar-exp latency
            for qg in range(1, TG):
                attn_block(1, 0, qg)
                proj_chunk(qg - 1, 0, 8)
                attn_block(1, 1, qg)
                if qg < 3:
                    proj_chunk(qg - 1, 8, 16)
            proj_chunk(2, 8, 16, alt=True)
            proj_chunk(3, 0, 16, alt=True)

    nc.finalize()
    return nc


def _host_inputs(x, freqs_cis, w_attn, w_proj):
    """Build the 8 per-core input maps (bf16 storage)."""
    bf16 = ml_dtypes.bfloat16
    x = np.asarray(x, dtype=np.float32)
    freqs_cis = np.asarray(freqs_cis, dtype=np.float32)
    w_attn = np.asarray(w_attn, dtype=np.float32)
    w_proj = np.asarray(w_proj, dtype=np.float32)

    B = x.shape[0]
    perm = np.concatenate([np.arange(0, HSIZE, 2), np.arange(1, HSIZE, 2)])

    cos = np.ascontiguousarray(freqs_cis[:, :, 0].T)  # [64, T]
    sin = np.ascontiguousarray(freqs_cis[:, :, 1].T)
    c1 = np.concatenate([cos, cos], axis=0).astype(bf16)     # [128, T]
    c2 = np.concatenate([-sin, sin], axis=0).astype(bf16)

    kk = np.arange(P)[:, None]
    ccol = np.arange(QG)[None, :]
    mk = np.stack(
        [(ccol >= s * 128 + kk).astype(np.float32) for s in range(2)], axis=0
    ).astype(bf16)  # [2,128,512]

    ones_col = np.ones((P, 1), bf16)

    xT = [np.ascontiguousarray(x[b].T).astype(bf16) for b in range(B)]

    in_maps = []
    for core in range(N_CORES):
        b, g = core // 4, core % 4
        blocks = []
        for pairp in range(2):
            for off in (0, C, 2 * C):  # q, k, v origins in w_attn
                for hh in range(2):
                    hglob = 4 * g + 2 * pairp + hh
                    cols = w_attn[:, off + hglob * HSIZE: off + (hglob + 1) * HSIZE]
                    if off != 2 * C:  # permute q and k, not v
                        cols = cols[:, perm]
                    blocks.append(cols)
        wcore = np.ascontiguousarray(np.concatenate(blocks, axis=1)).astype(bf16)
        wpcore = np.ascontiguousarray(w_proj[g * 512:(g + 1) * 512, :]).astype(bf16)
        in_maps.append(
            {
                "xT": xT[b],
                "w": wcore,
                "wp": wpcore,
                "c1": c1,
                "c2": c2,
                "mk": mk,
                "ones_col": ones_col,
            }
        )
    return in_maps


_LAST_RESULT = {}


def kernel(x, freqs_cis, w_attn, w_proj):
    if _TRACE:
        _install_ntff_hook()
    in_maps = _host_inputs(x, freqs_cis, w_attn, w_proj)
    nc = build_nc()
    res = run_bass_kernel_spmd(
        nc, in_maps, core_ids=list(range(N_CORES)), trace=_TRACE
    )
    _LAST_RESULT["res"] = res

    B = x.shape[0]
    out = np.zeros((B, T, C), dtype=np.float32)
    for core in range(N_CORES):
        b = core // 4
        out[b] += res.results[core]["outT"].astype(np.float32).T
    return out


# revision 61
# speedup vs baseline: 1.0281x; 1.0281x over previous
"""Causal self-attention (dense transformer block) for 8 Trainium2 NeuronCores.

Sharding: DP over batch (2) x TP over heads (4 groups of 4 heads) = 8 cores.
Per core: column-parallel QKV projection (4 heads), RoPE, causal
flash-attention (no-max-subtraction softmax with constant bias), row-parallel
output projection producing a partial [oc, t] result in fp16; host sums the
4 TP partials per batch and transposes back.

v2 design vs baseline:
- bf16 storage for x/w/q/k/v/yt/p (PSUM accumulation stays f32): halves DMA
  and SBUF, so x stays resident across both head-pairs (no re-read, no yt
  DRAM spill) and the output DMA is fp16.
- Softmax denominators: p tiles reduce via pair/quad DVE adds (independent
  bf16 2x ops) + one short ones-matmul per pair/quad, instead of a full
  512-cycle PE pass per kt tile.
- RoPE's rotate-by-64-partitions runs as two SBUF->SBUF DMAs on idle DMA
  engines instead of a PE permutation matmul.
- Fine-grained causal diagonal: the last two kt tiles of each diagonal block
  compute only q columns 256:512 (the rest is fully masked).
- Phase interleaving: pair-1 QKV matmuls are emitted between pair-0 attention
  blocks, and the output projection trails pair-1 attention by one q-group,
  so scalar-engine exps always hide under dense PE matmul streams.
- Startup: warmup matmuls run on memset tiles (no DMA dependency), input DMAs
  are issued in fine (w,x) chunk pairs consumed by a cc-major first t-group,
  with keepalive matmuls keeping the PE's HAM clock-gate warm through the
  DMA ramp.

Self-contained: hardcodes shapes, builds/compiles/runs the Bass kernel via
run_bass_kernel_spmd on cores 0-7.
"""

import os
import sys
import types

sys.path.insert(0, "/opt/trn_rl_repo")

import numpy as np
import ml_dtypes

import concourse.bass as bass
import concourse.mybir as mybir
import concourse.tile as tile
from concourse import bacc
from concourse.bass_utils import run_bass_kernel_spmd
from concourse.vector_clock import ScopedClock, VectorClock

F32 = mybir.dt.float32
F32R = mybir.dt.float32r
BF16 = mybir.dt.bfloat16
F16 = mybir.dt.float16
AF = mybir.ActivationFunctionType
ALU = mybir.AluOpType

P = 128
T = 2048
C = 2048
NH = 16          # total heads
HPC = 4          # heads per core
HSIZE = 128
N_CORES = 8
TG = 4           # t-groups of 512
QG = 512
EXP_BIAS = -10.0
SCALE = 1.0 / float(np.sqrt(HSIZE))

_TRACE = os.environ.get("BASS_KERNEL_TRACE", "0") == "1"


def _patch_tile_drain():
    """walrus in this toolchain allows at most one sync-wait per instruction;
    TileContext's tail drain aggregates the whole global clock onto one Drain.
    Split it: one Drain per pending proc, each with a single wait."""
    if getattr(tile.TileContext, "_drain_patched", False):
        return

    def _drain_and_barrier(self, tick_clock, wait_clock):
        nc = self.nc
        gc = tick_clock.global_clock
        n = len(gc)
        for p in range(n):
            if gc[p] > 0:
                vc = VectorClock([gc[p] if i == p else 0 for i in range(n)])
                di = nc.sync.drain()
                wait_clock.add_sem_waits(di.ins, ScopedClock({None: vc}))
        nc.all_engine_barrier()
        popped = nc._tile_sem_poison_stack.pop()
        assert popped is self._sem_poison
        nc.clear_and_free_semaphores(list(self.sems.allocated().values()))
        nc.all_engine_barrier()

    tile.TileContext._drain_and_barrier = _drain_and_barrier
    tile.TileContext._drain_patched = True


def _install_ntff_hook():
    """Wire the axon NTFF profiling hook this image leaves unwired (the agent
    image's antenv lacks axon_hooks). Only needed when tracing."""
    import antenv

    if getattr(antenv, "axon_hooks", None) is not None:
        return
    mod = types.ModuleType("antenv.axon_hooks")
    mod._hook = None
    mod.set_axon_ntff_profile_hook = lambda h: setattr(mod, "_hook", h)
    mod.get_axon_ntff_profile_hook = lambda: mod._hook
    sys.modules["antenv.axon_hooks"] = mod
    antenv.axon_hooks = mod
    if "/root/.axon_site" not in sys.path:
        sys.path.insert(0, "/root/.axon_site")
    try:
        from trn_agent_boot.trn_boot import _ntff_profile_via_ctypes

        hook = _ntff_profile_via_ctypes("/opt/axon/libaxon_pjrt.so")
        if hook is not None:
            mod.set_axon_ntff_profile_hook(hook)
        import concourse.bass_utils as bu

        bu.upload_artifacts = lambda d: d
    except Exception:
        pass


def build_nc():
    _patch_tile_drain()
    nc = bacc.Bacc(None, target_bir_lowering=False)

    xT = nc.dram_tensor("xT", [C, T], BF16, kind="ExternalInput")
    w = nc.dram_tensor("w", [C, 6 * HSIZE * 2], BF16, kind="ExternalInput")
    wp = nc.dram_tensor("wp", [HPC * HSIZE, T], BF16, kind="ExternalInput")
    c1d = nc.dram_tensor("c1", [P, T], BF16, kind="ExternalInput")
    c2d = nc.dram_tensor("c2", [P, T], BF16, kind="ExternalInput")
    mkd = nc.dram_tensor("mk", [2, P, QG], BF16, kind="ExternalInput")
    onesd = nc.dram_tensor("ones_col", [P, 1], BF16, kind="ExternalInput")
    outT = nc.dram_tensor("outT", [T, T], F16, kind="ExternalOutput")  # [oc, t]

    xTr = xT.rearrange("(cc p) t -> p cc t", p=P)      # [128,16,2048]
    wr = w.rearrange("(cc p) j -> p cc j", p=P)        # [128,16,1536]
    wpr = wp.rearrange("(hc p) t -> p hc t", p=P)      # [128,4,2048]
    mkr = mkd.rearrange("s p q -> p s q")              # [128,2,512]

    with tile.TileContext(nc) as tc, nc.allow_low_precision(
        reason="bf16 storage / f32 accumulation is the intended format"
    ):
        with (
            tc.tile_pool(name="const", bufs=1) as constp,
            tc.tile_pool(name="xpool", bufs=1) as xpool,
            tc.tile_pool(name="wpool", bufs=1) as wpool,
            tc.tile_pool(name="qk", bufs=1) as qkres,
            tc.tile_pool(name="vres", bufs=1) as vresp,
            tc.tile_pool(name="yres", bufs=1) as yresp,
            tc.tile_pool(name="work", bufs=5) as work,
            tc.tile_pool(name="pwork", bufs=6) as pwork,
            tc.tile_pool(name="ppp", bufs=2) as pppool,
            tc.tile_pool(name="yp", bufs=2) as yprep,
            tc.tile_pool(name="rp", bufs=2) as rpool,
            tc.tile_pool(name="rbig", bufs=1) as rbigp,
            tc.tile_pool(name="stg", bufs=6) as stgp,
            tc.tile_pool(name="mm", bufs=3, space="PSUM") as mmp,
            tc.tile_pool(name="yt", bufs=2, space="PSUM") as ytp,
            tc.tile_pool(name="lp", bufs=1, space="PSUM") as lpp,
        ):
            # ---- resident tiles ----
            x_sb = xpool.tile([P, 16, T], BF16, tag="x")          # 8.39MB
            c1 = constp.tile([P, T], BF16, tag="c1")
            c2 = constp.tile([P, T], BF16, tag="c2")
            mk = constp.tile([P, 2, QG], BF16, tag="mk")
            ones_c = constp.tile([P, 1], BF16, tag="onc")
            ebias = constp.tile([P, 1], F32, tag="ebias")
            wu_st = constp.tile([P, P], BF16, tag="wu_st")
            wu_mv = constp.tile([P, 128], BF16, tag="wu_mv")
            # q/k per pair per head: separate tags so pair1 QKV never waits
            # on pair0 attention reads
            q_sb = [[qkres.tile([P, T], BF16, tag=f"q{pr}{h}", name=f"q{pr}{h}")
                     for h in range(2)] for pr in range(2)]
            k_sb = [[qkres.tile([P, T], BF16, tag=f"k{pr}{h}", name=f"k{pr}{h}")
                     for h in range(2)] for pr in range(2)]
            # v per pair: pair-1 v writes must not alias pair-0's values
            # while pair-0 attention is still reading them
            v_sb = [vresp.tile([P, 16, 256], BF16, tag=f"v{pr}", name=f"v{pr}")
                    for pr in range(2)]
            yt_sb = [yresp.tile([P, T], BF16, tag=f"yt{i}", name=f"yt{i}")
                     for i in range(4)]

            nc.vector.memset(ebias[:], EXP_BIAS)
            nc.vector.memset(wu_st[:], 0.125)
            nc.vector.memset(wu_mv[:], 0.125)

            # ---- PE warmup on memset tiles: no DMA dependency ----
            for i in range(42):
                ps_wu = mmp.tile([P, 128], F32, tag="mm", name="ps_wu")
                nc.tensor.matmul(ps_wu[:], wu_st[:], wu_mv[:],
                                 start=True, stop=True)

            def keepalive(ap128):
                """One tiny matmul reading a landed DMA chunk: keeps the HAM
                activity window busy during the startup DMA ramp."""
                ps_ka = mmp.tile([P, P], F32, tag="mm", name="ps_ka")
                nc.tensor.matmul(ps_ka[:], wu_st[:], ap128, start=True,
                                 stop=True)

            # ---- startup DMAs: per-cc-chunk (w, x) interleaved so the
            # cc-major tg0 QKV unlocks 8 matmuls per landed chunk ----
            w_sb = wpool.tile([P, 16, 768], BF16, tag="w", name="w_sb0")
            for ch in range(8):
                nc.sync.dma_start(
                    w_sb[:, ch * 2:(ch + 1) * 2, :],
                    wr[:, ch * 2:(ch + 1) * 2, 0:768],
                )
                keepalive(w_sb[:, ch * 2, 0:128])
                nc.sync.dma_start(
                    x_sb[:, ch * 2:(ch + 1) * 2, 0:QG],
                    xTr[:, ch * 2:(ch + 1) * 2, 0:QG],
                )
                keepalive(x_sb[:, ch * 2, 0:128])
            nc.sync.dma_start(c1[:], c1d[:])
            nc.sync.dma_start(c2[:], c2d[:])
            for tg in range(1, TG):
                for ch in range(4):
                    nc.sync.dma_start(
                        x_sb[:, ch * 4:(ch + 1) * 4, tg * QG:(tg + 1) * QG],
                        xTr[:, ch * 4:(ch + 1) * 4, tg * QG:(tg + 1) * QG],
                    )
                if tg == 1:
                    keepalive(x_sb[:, 0, QG:QG + 128])
            nc.sync.dma_start(ones_c[:], onesd[:])
            nc.sync.dma_start(mk[:], mkr)
            # proj weights: own tag, loaded up front so phase C never waits
            wp_sb = wpool.tile([P, 4, T], BF16, tag="wpj", name="wp_sb")
            for ocq in range(4):
                nc.sync.dma_start(
                    wp_sb[:, :, ocq * 512:(ocq + 1) * 512],
                    wpr[:, :, ocq * 512:(ocq + 1) * 512],
                )

            # ---------- building blocks ----------

            def rope(pair, j, psum, tg):
                """psum [128,512] raw q/k j-tile -> roped bf16 into q/k_sb.
                j: 0,1 = q heads 0,1; 2,3 = k heads 0,1. The rotate-by-64
                partition swap runs as two SBUF->SBUF DMAs (idle DMA engines)
                instead of a PE permutation matmul; DVE ops stay pure-bf16."""
                dst = (q_sb[pair] if j < 2 else k_sb[pair])[j % 2]
                dsl = dst[:, tg * QG:(tg + 1) * QG]
                qraw = work.tile([P, QG], BF16, tag="tmp", name="qraw")
                nc.scalar.activation(qraw[:], psum[:], AF.Copy)
                qs = work.tile([P, QG], BF16, tag="tmp", name="qs")
                nc.sync.dma_start(qs[0:64, :], qraw[64:128, :])
                nc.sync.dma_start(qs[64:128, :], qraw[0:64, :])
                t1 = work.tile([P, QG], BF16, tag="tmp", name="t1")
                t2 = work.tile([P, QG], BF16, tag="tmp", name="t2")
                c1s = c1[:, tg * QG:(tg + 1) * QG]
                c2s = c2[:, tg * QG:(tg + 1) * QG]
                nc.vector.tensor_mul(t1[:], qraw[:], c1s)
                nc.vector.tensor_mul(t2[:], qs[:], c2s)
                nc.vector.tensor_add(dsl, t1[:], t2[:])

            def qkv_jp_unit(pair, tg, jp):
                """One (tg, jp) QKV unit: 2 psum accumulations over 16 cc,
                then rope both j-tiles."""
                psq = [mmp.tile([P, QG], F32, tag="psq", bufs=2,
                                name=f"psq{j}")
                       for j in range(2)]
                for cc in range(16):
                    xs = x_sb[:, cc, tg * QG:(tg + 1) * QG]
                    for j in range(2):
                        nc.tensor.matmul(
                            psq[j][:],
                            w_sb[:, cc, (jp * 2 + j) * 128:(jp * 2 + j + 1) * 128],
                            xs,
                            start=(cc == 0),
                            stop=(cc == 15),
                        )
                rope(pair, jp * 2 + 0, psq[0], tg)
                rope(pair, jp * 2 + 1, psq[1], tg)

            def qkv_v_unit(pair, tg):
                """v for one tg: 4 t-tiles, N=256 (both heads)."""
                for tt in range(4):
                    psv = ytp.tile([P, 256], F32, tag="yt", name="psv")
                    for cc in range(16):
                        nc.tensor.matmul(
                            psv[:],
                            x_sb[:, cc, (tg * 4 + tt) * 128:(tg * 4 + tt + 1) * 128],
                            w_sb[:, cc, 512:768],
                            start=(cc == 0),
                            stop=(cc == 15),
                        )
                    nc.vector.tensor_copy(v_sb[pair][:, tg * 4 + tt, :],
                                          psv[:])

            def attn_block(pair, h, qg):
                """Flash-attention block for one (head, q-group). Softmax
                denominator: pairwise p-tile adds on DVE (independent bf16
                2x ops) + one small ones-matmul per pair of kt tiles."""
                hg = pair * 2 + h
                n_kt = 4 * qg + 4
                LA = 3
                ps_y = ytp.tile([P, QG], F32, tag="yt", name="ps_y")
                ps_l = lpp.tile([1, QG], F32, tag="l", name="ps_l")
                p_tiles = {}
                pp_pend = [None]
                lfirst = [True]

                def emit_s(kt):
                    s = kt - 4 * qg
                    # last two diagonal tiles: q cols [0,256) are entirely
                    # masked, so compute only the upper half width
                    half = s >= 2
                    qo = 256 if half else 0
                    w = 256 if half else QG
                    ps_s = mmp.tile([P, w], F32, tag="mm", name="ps_s")
                    nc.tensor.matmul(
                        ps_s[:],
                        k_sb[pair][h][:, kt * 128:(kt + 1) * 128],
                        q_sb[pair][h][:, qg * QG + qo:qg * QG + qo + w],
                        start=True,
                        stop=True,
                    )
                    p_sb = pwork.tile([P, w], BF16, tag="p", name="p_sb")
                    nc.scalar.activation(
                        p_sb[:], ps_s[:], AF.Exp, bias=ebias[:], scale=SCALE
                    )
                    if s >= 0:
                        # half tiles (s=2,3) at q cols 256:512 follow the
                        # same triangular pattern as s-2 at cols 0:256
                        ms, mo = (s - 2, 0) if half else (s, qo)
                        nc.vector.tensor_mul(p_sb[:], p_sb[:],
                                             mk[:, ms, mo:mo + w])
                    p_tiles[kt] = p_sb
                    if kt % 2 == 1:
                        # pair partner always has the same width: the two
                        # half tiles are the final (even, odd) kt pair
                        pp = pppool.tile([P, w], BF16, tag="pp", name="pp")
                        nc.vector.tensor_add(
                            pp[:], p_tiles[kt - 1][:], p_sb[:]
                        )
                        # full-width pairs combine into quads (one DVE add
                        # replaces a PE ones-matmul); the final mixed-width
                        # quad keeps two matmuls
                        if w == QG and kt % 4 == 1 and kt + 2 < n_kt - 1:
                            pp_pend[0] = pp
                            return
                        if w == QG and kt % 4 == 3 and pp_pend[0] is not None:
                            nc.vector.tensor_add(pp[:], pp_pend[0][:], pp[:])
                            pp_pend[0] = None
                        nc.tensor.matmul(
                            ps_l[0:1, qo:qo + w], ones_c[:], pp[:],
                            start=lfirst[0], stop=(kt == n_kt - 1),
                            skip_group_check=True,
                        )
                        lfirst[0] = False

                def emit_av(kt):
                    half = (kt - 4 * qg) >= 2
                    qo = 256 if half else 0
                    w = 256 if half else QG
                    p_sb = p_tiles.pop(kt)
                    nc.tensor.matmul(
                        ps_y[:, qo:qo + w],
                        v_sb[pair][:, kt, h * 128:(h + 1) * 128],
                        p_sb[:],
                        start=(kt == 0),
                        stop=(kt == n_kt - 1),
                        skip_group_check=True,
                    )

                for kt in range(n_kt + LA):
                    if kt < n_kt:
                        emit_s(kt)
                    if kt >= LA:
                        emit_av(kt - LA)

                # free ps_y immediately; normalize later in pure bf16
                ypre = yprep.tile([P, QG], BF16, tag="ypre", name="ypre")
                nc.vector.tensor_copy(ypre[:], ps_y[:])
                with tc.high_priority(offset=40):
                    r_f32 = rpool.tile([1, QG], F32, tag="rf", name="r_f32")
                    nc.vector.reciprocal_approx_fast(r_f32[:], ps_l[:])
                    r_bf = rpool.tile([1, QG], BF16, tag="rb", name="r_bf")
                    nc.vector.tensor_copy(r_bf[:], r_f32[:])
                r128 = rbigp.tile([P, QG], BF16, tag="r128", name="r128")
                nc.gpsimd.partition_broadcast(r128[:], r_bf[0:1, :])
                nc.vector.tensor_mul(
                    yt_sb[hg][:, qg * QG:(qg + 1) * QG], ypre[:], r128[:]
                )

            def proj_chunk(tg, oc_lo, oc_hi, alt=False):
                """Output projection tiles [oc_lo, oc_hi) for one t-group.
                oc tiles run in pairs (two concurrent psum groups) so each
                psum slot cycle carries ~1.7us of PE work, hiding the
                stage-copy latency; stage copies alternate scalar/DVE.
                alt=True (proj-only tail): alternate pair-groups between the
                psq and mm tags for 5 effective slots."""
                for gi, ocp in enumerate(range(oc_lo // 2, oc_hi // 2)):
                    if alt and gi % 2 == 1:
                        ps_o = [mmp.tile([P, QG], F32, tag="mm",
                                         name=f"ps_o{j}") for j in range(2)]
                    else:
                        ps_o = [mmp.tile([P, QG], F32, tag="psq", bufs=2,
                                         name=f"ps_o{j}") for j in range(2)]
                    for hc in range(4):
                        for j in range(2):
                            oc = ocp * 2 + j
                            nc.tensor.matmul(
                                ps_o[j][:],
                                wp_sb[:, hc, oc * 128:(oc + 1) * 128],
                                yt_sb[hc][:, tg * QG:(tg + 1) * QG],
                                start=(hc == 0),
                                stop=(hc == 3),
                            )
                    for j in range(2):
                        oc = ocp * 2 + j
                        stage = stgp.tile([P, QG], F16, tag="stg",
                                          name="stage")
                        if j == 0:
                            nc.scalar.activation(stage[:], ps_o[j][:],
                                                 AF.Copy)
                        else:
                            nc.vector.tensor_copy(stage[:], ps_o[j][:])
                        # spread out-DMAs over both HWDGE queues
                        dma_eng = nc.sync if j == 0 else nc.scalar
                        dma_eng.dma_start(
                            outT[oc * 128:(oc + 1) * 128,
                                 tg * QG:(tg + 1) * QG],
                            stage[:],
                        )

            # ---------- phase A: QKV pair 0 ----------
            # tg0 runs cc-major (all 6 psum groups at once) so PE work
            # unlocks progressively as each startup (w,x) chunk lands
            psq4 = [mmp.tile([P, QG], F32, tag="psq", bufs=2,
                             name=f"psq4_{j}") for j in range(2)]
            psq4 += [mmp.tile([P, QG], F32, tag="mm", name=f"psq4_{j}")
                     for j in range(2, 4)]
            for cc in range(16):
                xs = x_sb[:, cc, 0:QG]
                for j in range(4):
                    nc.tensor.matmul(
                        psq4[j][:], w_sb[:, cc, j * 128:(j + 1) * 128], xs,
                        start=(cc == 0), stop=(cc == 15),
                    )
            for j in range(4):
                rope(0, j, psq4[j], 0)
            qkv_v_unit(0, 0)
            for tg in range(1, TG):
                qkv_jp_unit(0, tg, 0)
                qkv_jp_unit(0, tg, 1)
                qkv_v_unit(0, tg)

            # pair-1 weights land during phase B (fine chunks so the first
            # QKV units' cc-consumption tracks DMA arrival)
            w_sb = wpool.tile([P, 16, 768], BF16, tag="w", name="w_sb1")
            for wcc in range(8):
                nc.sync.dma_start(
                    w_sb[:, wcc * 2:(wcc + 1) * 2, :],
                    wr[:, wcc * 2:(wcc + 1) * 2, 768:1536],
                )

            # ---------- phase B: attention pair 0 || QKV pair 1 ----------
            # attention blocks first while pair-1 weights land; QKV units
            # interleaved to keep PE fed while scalar runs exps
            b_sched = [
                ("a", 0, 0), ("a", 1, 0), ("a", 0, 1), ("a", 1, 1),
                ("q", 0, 0), ("v", 0, None), ("q", 1, 0), ("a", 0, 2),
                ("q", 2, 0), ("a", 1, 2), ("q", 3, 0), ("a", 0, 3),
                ("q", 0, 1), ("A", 0, 0), ("q", 1, 1), ("v", 1, None),
                ("A", 1, 0), ("q", 2, 1), ("a", 1, 3), ("q", 3, 1),
                ("v", 2, None), ("v", 3, None),
            ]
            for kind, i1, i2 in b_sched:
                if kind == "a":
                    attn_block(0, i1, i2)
                elif kind == "A":
                    attn_block(1, i1, i2)
                elif kind == "q":
                    qkv_jp_unit(1, i1, i2)
                else:
                    qkv_v_unit(1, i1)

            # ---------- phase C: attention pair 1 || projection ----------
            # proj chunks trail attention by one q-group so the final attn
            # blocks always have dense proj matmuls co-scheduled behind
            # their scalar-exp latency
            for qg in range(1, TG):
                attn_block(1, 0, qg)
                proj_chunk(qg - 1, 0, 8)
                attn_block(1, 1, qg)
                if qg < 3:
                    proj_chunk(qg - 1, 8, 16)
            proj_chunk(2, 8, 16, alt=True)
            proj_chunk(3, 0, 16, alt=True)

    nc.finalize()
    return nc


def _host_inputs(x, freqs_cis, w_attn, w_proj):
    """Build the 8 per-core input maps (bf16 storage)."""
    bf16 = ml_dtypes.bfloat16
    x = np.asarray(x, dtype=np.float32)
    freqs_cis = np.asarray(freqs_cis, dtype=np.float32)
    w_attn = np.asarray(w_attn, dtype=np.float32)
    w_proj = np.asarray(w_proj, dtype=np.float32)

    B = x.shape[0]
    perm = np.concatenate([np.arange(0, HSIZE, 2), np.arange(1, HSIZE, 2)])

    cos = np.ascontiguousarray(freqs_cis[:, :, 0].T)  # [64, T]
    sin = np.ascontiguousarray(freqs_cis[:, :, 1].T)
    c1 = np.concatenate([cos, cos], axis=0).astype(bf16)     # [128, T]
    c2 = np.concatenate([-sin, sin], axis=0).astype(bf16)

    kk = np.arange(P)[:, None]
    ccol = np.arange(QG)[None, :]
    mk = np.stack(
        [(ccol >= s * 128 + kk).astype(np.float32) for s in range(2)], axis=0
    ).astype(bf16)  # [2,128,512]

    ones_col = np.ones((P, 1), bf16)

    xT = [np.ascontiguousarray(x[b].T).astype(bf16) for b in range(B)]

    in_maps = []
    for core in range(N_CORES):
        b, g = core // 4, core % 4
        blocks = []
        for pairp in range(2):
            for off in (0, C, 2 * C):  # q, k, v origins in w_attn
                for hh in range(2):
                    hglob = 4 * g + 2 * pairp + hh
                    cols = w_attn[:, off + hglob * HSIZE: off + (hglob + 1) * HSIZE]
                    if off != 2 * C:  # permute q and k, not v
                        cols = cols[:, perm]
                    blocks.append(cols)
        wcore = np.ascontiguousarray(np.concatenate(blocks, axis=1)).astype(bf16)
        wpcore = np.ascontiguousarray(w_proj[g * 512:(g + 1) * 512, :]).astype(bf16)
        in_maps.append(
            {
                "xT": xT[b],
                "w": wcore,
                "wp": wpcore,
                "c1": c1,
                "c2": c2,
                "mk": mk,
                "ones_col": ones_col,
            }
        )
    return in_maps


_LAST_RESULT = {}


def kernel(x, freqs_cis, w_attn, w_proj):
    if _TRACE:
        _install_ntff_hook()
    in_maps = _host_inputs(x, freqs_cis, w_attn, w_proj)
    nc = build_nc()
    res = run_bass_kernel_spmd(
        nc, in_maps, core_ids=list(range(N_CORES)), trace=_TRACE
    )
    _LAST_RESULT["res"] = res

    B = x.shape[0]
    out = np.zeros((B, T, C), dtype=np.float32)
    for core in range(N_CORES):
        b = core // 4
        out[b] += res.results[core]["outT"].astype(np.float32).T
    return out
